# revision 1
# baseline (speedup 1.0000x reference)
"""Multi-head attention (B=2, S=2048, D=1024, H=16) on 8 Trainium2 NeuronCores.

Sharding: data-parallel over batch (2 groups of 4 cores) x tensor-parallel over
heads (4 heads / core). Each core computes its 4 heads' Q/K/V projections,
attention, and a partial output projection; the host sums the 4 partials per
batch and adds b_o.

Per-core device kernel layout notes:
  - All matmul operands are float32r (TF32-like, 1 cyc/row at N>=256).
  - Host passes q/k/v pre-transposed ([D, S]) so feature dim lands on
    partitions (matmul contracts along partitions).
  - Scores are computed transposed (S^T [k-tok, q-tok]) so softmax'd probs
    feed the PV matmul directly as the moving operand.
  - Softmax skips max-subtraction (scores ~ N(0,1), exp can't overflow).
  - The per-head denominator l = sum_k exp(S) is produced by augmenting the
    PV stationary operand V with a ones-column (M=65): psum row 64 = l.
  - Normalization: linv = 1/l (DVE), broadcast across partitions with a
    K=1 ones-row matmul, then fused multiply during the PSUM->SBUF copy.
  - Output projection computes out^T; host transposes back.
"""

import numpy as np

D_MODEL = 1024
S = 2048
N_CORES = 8
HPC = 4          # heads per core
COF = HPC * 64   # 256 out-features per core

_CACHED_NC = None


def _build():
    from concourse import bacc
    import concourse.bass as bass
    import concourse.tile as tile
    from concourse import mybir

    F32R = mybir.dt.float32r
    F32 = mybir.dt.float32
    EXP = mybir.ActivationFunctionType.Exp

    nc = bacc.Bacc("TRN2", target_bir_lowering=False, debug=False,
                   num_devices=N_CORES)

    qT = nc.dram_tensor("qT", [D_MODEL, S], F32R, kind="ExternalInput")
    kT = nc.dram_tensor("kT", [D_MODEL, S], F32R, kind="ExternalInput")
    vT = nc.dram_tensor("vT", [D_MODEL, S], F32R, kind="ExternalInput")
    wq = nc.dram_tensor("wq", [D_MODEL, COF], F32R, kind="ExternalInput")
    wk = nc.dram_tensor("wk", [D_MODEL, COF], F32R, kind="ExternalInput")
    wv = nc.dram_tensor("wv", [D_MODEL, COF], F32R, kind="ExternalInput")
    wo = nc.dram_tensor("wo", [COF, D_MODEL], F32R, kind="ExternalInput")
    bq2 = nc.dram_tensor("bq2", [128, 2], F32, kind="ExternalInput")
    bk2 = nc.dram_tensor("bk2", [128, 2], F32, kind="ExternalInput")
    bv4 = nc.dram_tensor("bv4", [HPC, 64], F32, kind="ExternalInput")
    ones = nc.dram_tensor("ones", [1, 64], F32R, kind="ExternalInput")
    outT = nc.dram_tensor("outT", [D_MODEL, S], F32, kind="ExternalOutput")

    with nc.allow_low_precision(reason="float32r matmul rounding is intended"), \
            tile.TileContext(nc) as tc:
        with (
            tc.tile_pool(name="wconst", bufs=1) as wconst,
            tc.tile_pool(name="big", bufs=1) as big,
            tc.tile_pool(name="qin", bufs=3) as qin_pool,
            tc.tile_pool(name="expp", bufs=4) as expp,
            tc.tile_pool(name="stage", bufs=3) as stage_pool,
            tc.tile_pool(name="bcp", bufs=2) as bcp,
            tc.tile_pool(name="small", bufs=4) as small,
            tc.tile_pool(name="psA", bufs=4, space="PSUM") as psA,
            tc.tile_pool(name="psS", bufs=2, space="PSUM") as psS,
        ):
            # ---- constants ----
            wq_sb = wconst.tile([128, 8, COF], F32R)
            wk_sb = wconst.tile([128, 8, COF], F32R)
            wv_sb = wconst.tile([128, 8, COF], F32R)
            wo_sb = wconst.tile([128, 2, D_MODEL], F32R)
            nc.sync.dma_start(wq_sb[:], wq[:].rearrange("(a p) f -> p a f", p=128))
            nc.sync.dma_start(wk_sb[:], wk[:].rearrange("(a p) f -> p a f", p=128))
            nc.sync.dma_start(wv_sb[:], wv[:].rearrange("(a p) f -> p a f", p=128))
            nc.sync.dma_start(wo_sb[:], wo[:].rearrange("(c p) f -> p c f", p=128))
            bq_sb = wconst.tile([128, 2], F32)
            bk_sb = wconst.tile([128, 2], F32)
            nc.sync.dma_start(bq_sb[:], bq2[:])
            nc.sync.dma_start(bk_sb[:], bk2[:])
            bv_bc = wconst.tile([128, HPC, 64], F32)
            bv_ap = bv4[:]
            nc.gpsimd.dma_start(
                bv_bc[:],
                bass.AP(tensor=bv_ap.tensor, offset=bv_ap.offset,
                        ap=[[0, 128], [64, HPC], [1, 64]]),
            )
            ones_sb = wconst.tile([1, 64], F32R)
            nc.sync.dma_start(ones_sb[:], ones[:])

            # ---- persistent activations ----
            QT_sb = big.tile([128, 2, S], F32R)   # [p, m, t]: Q^T[m*128+p, t]
            KT_sb = big.tile([128, 2, S], F32R)
            V_sb = big.tile([128, 16, HPC, 65], F32R)  # [tok%128, tok//128, h, c]
            OT_sb = big.tile([128, 2, S], F32R)   # normalized attention out^T

            # V ones-column (l accumulator rides along the PV matmul)
            ones_ap = ones[:]
            for tt in range(16):
                nc.gpsimd.dma_start(
                    V_sb[:, tt, :, 64:65],
                    bass.AP(tensor=ones_ap.tensor, offset=ones_ap.offset,
                            ap=[[0, 128], [0, HPC], [1, 1]]),
                )

            # ---- projections ----
            # Chunk-interleaved so attention (which consumes K/V/Q in k-token
            # order) can start as soon as the first chunks are projected.
            def proj_qk_chunk(w_sb, b_sb, xT, dst, qc, pfx):
                # psum[of 128, tok 512] = sum_kt w[:,kt,of].T @ xT[kt, tok]
                xin = qin_pool.tile([128, 8, 512], F32R, tag="xin",
                                    name=f"{pfx}in_{qc}")
                nc.sync.dma_start(
                    xin[:],
                    xT[:].rearrange("(a p) t -> p a t", p=128)[
                        :, :, qc * 512:(qc + 1) * 512],
                )
                for m in range(2):
                    pq = psS.tile([128, 1024], F32, tag="sc",
                                  name=f"{pfx}ps_{qc}_{m}")
                    for kt in range(8):
                        nc.tensor.matmul(
                            pq[:, 0:512],
                            w_sb[:, kt, m * 128:(m + 1) * 128],
                            xin[:, kt, :],
                            start=(kt == 0), stop=(kt == 7),
                        )
                    nc.vector.tensor_scalar_add(
                        dst[:, m, qc * 512:(qc + 1) * 512], pq[:, 0:512],
                        b_sb[:, m:m + 1],
                    )

            def proj_v_chunk(vc):
                # psum[tok 128, of 256] = sum_kt vT[kt, tok].T @ wv[:, kt, :]
                vin = qin_pool.tile([128, 8, 512], F32R, tag="xin",
                                    name=f"vin_{vc}")
                nc.sync.dma_start(
                    vin[:],
                    vT[:].rearrange("(a p) t -> p a t", p=128)[
                        :, :, vc * 512:(vc + 1) * 512],
                )
                for tsub in range(4):
                    tt = vc * 4 + tsub
                    pv = psS.tile([128, 1024], F32, tag="sc",
                                  name=f"vps_{vc}_{tsub}")
                    for kt in range(8):
                        nc.tensor.matmul(
                            pv[:, 0:COF],
                            vin[:, kt, tsub * 128:(tsub + 1) * 128],
                            wv_sb[:, kt, :],
                            start=(kt == 0), stop=(kt == 7),
                        )
                    nc.vector.tensor_add(
                        V_sb[:, tt, :, 0:64],
                        pv[:, 0:COF].rearrange("p (h c) -> p h c", h=HPC),
                        bv_bc[:],
                    )

            # ---- attention helpers ----
            def att_pass_alloc(hp, qh):
                return [[psA.tile([128, 512], F32, tag="ps",
                                  name=f"po_{hp}_{qh}_{h2}_{qcl}")
                         for qcl in range(2)] for h2 in range(2)]

            def att_ktgroup(hp, qh, po, kts):
                for kt in kts:
                    for h2 in range(2):
                        p0 = h2 * 64
                        sc = psS.tile([128, 1024], F32, tag="sc",
                                      name=f"sc_{hp}_{qh}_{kt}_{h2}")
                        for qcl in range(2):
                            qg = qh * 2 + qcl
                            nc.tensor.matmul(
                                sc[:, qcl * 512:(qcl + 1) * 512],
                                KT_sb[p0:p0 + 64, hp, kt * 128:(kt + 1) * 128],
                                QT_sb[p0:p0 + 64, hp, qg * 512:(qg + 1) * 512],
                                start=True, stop=True,
                                tile_position=(p0, 0),
                            )
                        ex = expp.tile([128, 1024], F32R, tag="ex",
                                       name=f"ex_{hp}_{qh}_{kt}_{h2}")
                        nc.scalar.activation(out=ex[:], in_=sc[:], func=EXP,
                                             scale=0.125)
                        for qcl in range(2):
                            nc.tensor.matmul(
                                po[h2][qcl][0:65, :],
                                V_sb[:, kt, hp * 2 + h2, :],
                                ex[:, qcl * 512:(qcl + 1) * 512],
                                start=(kt == 0), stop=(kt == 15),
                            )

            def att_norm(hp, qh, po):
                # OT = po[0:64] / l  (l rides in po row 64)
                for h2 in range(2):
                    for qcl in range(2):
                        qg = qh * 2 + qcl
                        p = po[h2][qcl]
                        linv = small.tile([1, 512], F32R, tag="linv",
                                          name=f"linv_{hp}_{qh}_{h2}_{qcl}")
                        nc.vector.reciprocal(linv[:], p[64:65, :])
                        bc_ps = psS.tile([64, 512], F32, tag="sc",
                                         name=f"bc_{hp}_{qh}_{h2}_{qcl}")
                        nc.tensor.matmul(
                            bc_ps[:], ones_sb[:], linv[:],
                            start=True, stop=True,
                        )
                        bc_sb = bcp.tile([64, 512], F32, tag="bc",
                                         name=f"bcs_{hp}_{qh}_{h2}_{qcl}")
                        nc.vector.tensor_copy(bc_sb[:], bc_ps[:])
                        nc.vector.tensor_mul(
                            OT_sb[h2 * 64:(h2 + 1) * 64, hp,
                                  qg * 512:(qg + 1) * 512],
                            p[0:64, :], bc_sb[:],
                        )

            def outproj_half(qh):
                # out^T[of, t] = wo[:, of].T @ OT[:, t], token half qh
                for oft in range(8):
                    pg = [psA.tile([128, 512], F32, tag="ps",
                                   name=f"pg_{qh}_{oft}_{i}") for i in range(2)]
                    for ct in range(2):
                        for i in range(2):
                            tcn = qh * 2 + i
                            nc.tensor.matmul(
                                pg[i][:],
                                wo_sb[:, ct, oft * 128:(oft + 1) * 128],
                                OT_sb[:, ct, tcn * 512:(tcn + 1) * 512],
                                start=(ct == 0), stop=(ct == 1),
                            )
                    for i in range(2):
                        tcn = qh * 2 + i
                        st = stage_pool.tile([128, 512], F32, tag="st",
                                             name=f"st_{qh}_{oft}_{i}")
                        nc.vector.tensor_copy(st[:], pg[i][:])
                        nc.sync.dma_start(
                            outT[oft * 128:(oft + 1) * 128,
                                 tcn * 512:(tcn + 1) * 512],
                            st[:],
                        )

            # ---- schedule ----
            # Tile's static per-engine order follows program order, so ready
            # attention work must precede DMA-gated projection work: run pass
            # (hp0, qh0) kt-groups between the remaining input chunks.
            proj_qk_chunk(wk_sb, bk_sb, kT, KT_sb, 0, "k")
            proj_v_chunk(0)
            proj_qk_chunk(wq_sb, bq_sb, qT, QT_sb, 0, "q")
            proj_qk_chunk(wq_sb, bq_sb, qT, QT_sb, 1, "q")
            po00 = att_pass_alloc(0, 0)
            att_ktgroup(0, 0, po00, range(0, 4))
            proj_qk_chunk(wk_sb, bk_sb, kT, KT_sb, 1, "k")
            proj_v_chunk(1)
            att_ktgroup(0, 0, po00, range(4, 8))
            proj_qk_chunk(wk_sb, bk_sb, kT, KT_sb, 2, "k")
            proj_v_chunk(2)
            att_ktgroup(0, 0, po00, range(8, 12))
            proj_qk_chunk(wk_sb, bk_sb, kT, KT_sb, 3, "k")
            proj_v_chunk(3)
            att_ktgroup(0, 0, po00, range(12, 16))
            proj_qk_chunk(wq_sb, bq_sb, qT, QT_sb, 2, "q")
            proj_qk_chunk(wq_sb, bq_sb, qT, QT_sb, 3, "q")
            att_norm(0, 0, po00)

            po10 = att_pass_alloc(1, 0)
            att_ktgroup(1, 0, po10, range(16))
            att_norm(1, 0, po10)
            outproj_half(0)

            po01 = att_pass_alloc(0, 1)
            att_ktgroup(0, 1, po01, range(16))
            att_norm(0, 1, po01)
            po11 = att_pass_alloc(1, 1)
            att_ktgroup(1, 1, po11, range(16))
            att_norm(1, 1, po11)
            outproj_half(1)

    nc.compile()
    return nc


def _get_nc():
    global _CACHED_NC
    if _CACHED_NC is None:
        _CACHED_NC = _build()
    return _CACHED_NC


def kernel(q, k, v, w_q, b_q, w_k, b_k, w_v, b_v, w_o, b_o):
    from concourse.bass_utils import run_bass_kernel_spmd

    q, k, v = (np.asarray(x, np.float32) for x in (q, k, v))
    w_q, b_q, w_k, b_k, w_v, b_v, w_o, b_o = (
        np.asarray(x, np.float32)
        for x in (w_q, b_q, w_k, b_k, w_v, b_v, w_o, b_o)
    )

    nc = _get_nc()
    ones = np.ones((1, 64), np.float32)
    in_maps = []
    for core in range(N_CORES):
        b, hg = divmod(core, 4)
        sl = slice(hg * COF, (hg + 1) * COF)
        in_maps.append({
            "qT": np.ascontiguousarray(q[b].T),
            "kT": np.ascontiguousarray(k[b].T),
            "vT": np.ascontiguousarray(v[b].T),
            "wq": np.ascontiguousarray(w_q[:, sl]),
            "wk": np.ascontiguousarray(w_k[:, sl]),
            "wv": np.ascontiguousarray(w_v[:, sl]),
            "wo": np.ascontiguousarray(w_o[sl, :]),
            "bq2": b_q[sl].reshape(2, 128).T.copy(),
            "bk2": b_k[sl].reshape(2, 128).T.copy(),
            "bv4": b_v[sl].reshape(HPC, 64).copy(),
            "ones": ones,
        })

    res = run_bass_kernel_spmd(nc, in_maps, list(range(N_CORES)))
    out = np.zeros((2, S, D_MODEL), np.float32)
    for core in range(N_CORES):
        out[core // 4] += res.results[core]["outT"].T
    out += b_o
    return out



# revision 4
# speedup vs baseline: 11.8796x; 11.8796x over previous
"""Multi-head attention (B=2, S=2048, D=1024, H=16) on 8 Trainium2 NeuronCores.

Sharding: data-parallel over batch (2 groups of 4 cores) x tensor-parallel over
heads (4 heads / core). The wall clock is dominated by the host<->device tunnel,
so the design minimizes transferred bytes:

  - Inputs ship as fp16 token shards (each core gets 512 tokens of its batch,
    feature-major), then an on-device AllGather within each 4-core batch group
    reconstructs the full [1024, 2048] q/k/v — no host-side 4x replication.
  - Weights ship as fp16 per-head-group slices.
  - Each core's partial output projection is summed on device with a
    ReduceScatter, so each core returns only its 512 tokens of the final
    output in fp16 ([512, 1024]); the host does one contiguous cast + bias.
  - The jitted PJRT executable is built once and cached; repeat calls skip
    retracing and the in_maps/concat double copy.

Device kernel notes (per core):
  - Projection matmuls consume fp16 operands; attention internals stay f32r
    (same structure as the known-good baseline): scores computed transposed,
    softmax without max-subtraction, denominator via a ones-column in the PV
    stationary operand, 1/l broadcast with a K=1 ones matmul.
  - The output projection is computed token-major ([tok, out-feature] psum) so
    the ReduceScatter chunks are token-contiguous and the host reassembly is
    a plain reshape.
"""

import numpy as np

D_MODEL = 1024
S = 2048
N_CORES = 8
HPC = 4           # heads per core
COF = HPC * 64    # 256 out-features per core
TPS = S // 4      # 512 tokens per shard

_CACHED = None


def _build():
    from concourse import bacc
    import concourse.bass as bass
    import concourse.tile as tile
    from concourse import mybir

    F16 = mybir.dt.float16
    F32R = mybir.dt.float32r
    F32 = mybir.dt.float32
    EXP = mybir.ActivationFunctionType.Exp

    nc = bacc.Bacc("TRN2", target_bir_lowering=False, debug=False,
                   num_devices=N_CORES)

    xq = nc.dram_tensor("xq", [D_MODEL, TPS], F16, kind="ExternalInput")
    xk = nc.dram_tensor("xk", [D_MODEL, TPS], F16, kind="ExternalInput")
    xv = nc.dram_tensor("xv", [D_MODEL, TPS], F16, kind="ExternalInput")
    wq = nc.dram_tensor("wq", [D_MODEL, COF], F16, kind="ExternalInput")
    wk = nc.dram_tensor("wk", [D_MODEL, COF], F16, kind="ExternalInput")
    wv = nc.dram_tensor("wv", [D_MODEL, COF], F16, kind="ExternalInput")
    wo = nc.dram_tensor("wo", [COF, D_MODEL], F16, kind="ExternalInput")
    bq2 = nc.dram_tensor("bq2", [128, 2], F32, kind="ExternalInput")
    bk2 = nc.dram_tensor("bk2", [128, 2], F32, kind="ExternalInput")
    bv4 = nc.dram_tensor("bv4", [HPC, 64], F32, kind="ExternalInput")
    ones = nc.dram_tensor("ones", [1, 64], F32R, kind="ExternalInput")
    outp = nc.dram_tensor("outp", [TPS, D_MODEL], F16, kind="ExternalOutput")

    GROUPS4 = [[0, 1, 2, 3], [4, 5, 6, 7]]

    with nc.allow_low_precision(reason="fp16 transfers / f32r matmuls intended"), \
            tile.TileContext(nc) as tc:
        with (
            tc.tile_pool(name="dram", bufs=1, space="DRAM") as dram,
            tc.tile_pool(name="wconst", bufs=1) as wconst,
            tc.tile_pool(name="big", bufs=1) as big,
            tc.tile_pool(name="qin", bufs=2) as qin_pool,
            tc.tile_pool(name="expp", bufs=4) as expp,
            tc.tile_pool(name="stage", bufs=3) as stage_pool,
            tc.tile_pool(name="bcp", bufs=2) as bcp,
            tc.tile_pool(name="small", bufs=4) as small,
            tc.tile_pool(name="psA", bufs=4, space="PSUM") as psA,
            tc.tile_pool(name="psS", bufs=2, space="PSUM") as psS,
        ):
            # ---- collective bounce buffers (DRAM) ----
            xin_b = dram.tile([3, D_MODEL, TPS], F16)       # my shard of q,k,v
            gX = dram.tile([4, 3, D_MODEL, TPS], F16)       # gathered full seq
            ob_in = dram.tile([S, D_MODEL], F16)            # my partial out
            ob_out = dram.tile([TPS, D_MODEL], F16)         # reduced shard

            nc.sync.dma_start(xin_b[0], xq[:])
            nc.sync.dma_start(xin_b[1], xk[:])
            nc.sync.dma_start(xin_b[2], xv[:])
            nc.gpsimd.collective_compute(
                "AllGather",
                mybir.AluOpType.bypass,
                replica_groups=GROUPS4,
                ins=[xin_b[:].opt()],
                outs=[gX[:].opt()],
            )

            # ---- constants ----
            wq_sb = wconst.tile([128, 8, COF], F16)
            wk_sb = wconst.tile([128, 8, COF], F16)
            wv_sb = wconst.tile([128, 8, COF], F16)
            wo_sb = wconst.tile([128, 2, D_MODEL], F16)
            nc.sync.dma_start(wq_sb[:], wq[:].rearrange("(a p) f -> p a f", p=128))
            nc.sync.dma_start(wk_sb[:], wk[:].rearrange("(a p) f -> p a f", p=128))
            nc.sync.dma_start(wv_sb[:], wv[:].rearrange("(a p) f -> p a f", p=128))
            nc.sync.dma_start(wo_sb[:], wo[:].rearrange("(c p) f -> p c f", p=128))
            bq_sb = wconst.tile([128, 2], F32)
            bk_sb = wconst.tile([128, 2], F32)
            nc.sync.dma_start(bq_sb[:], bq2[:])
            nc.sync.dma_start(bk_sb[:], bk2[:])
            bv_bc = wconst.tile([128, HPC, 64], F32)
            bv_ap = bv4[:]
            nc.gpsimd.dma_start(
                bv_bc[:],
                bass.AP(tensor=bv_ap.tensor, offset=bv_ap.offset,
                        ap=[[0, 128], [64, HPC], [1, 64]]),
            )
            ones_sb = wconst.tile([1, 64], F32R)
            nc.sync.dma_start(ones_sb[:], ones[:])

            # ---- persistent activations ----
            QT_sb = big.tile([128, 2, S], F32R)   # [p, m, t]: Q^T[m*128+p, t]
            KT_sb = big.tile([128, 2, S], F32R)
            V_sb = big.tile([128, 16, HPC, 65], F32R)  # [tok%128, tok//128, h, c]
            OT_sb = big.tile([128, 2, S], F16)    # normalized attention out^T

            # V ones-column (l accumulator rides along the PV matmul)
            ones_ap = ones[:]
            for tt in range(16):
                nc.gpsimd.dma_start(
                    V_sb[:, tt, :, 64:65],
                    bass.AP(tensor=ones_ap.tensor, offset=ones_ap.offset,
                            ap=[[0, 128], [0, HPC], [1, 1]]),
                )

            # ---- projections (read gathered fp16 chunks) ----
            def proj_chunk(qc):
                xin = qin_pool.tile([128, 3, 8, TPS], F16, tag="xin",
                                    name=f"xin_{qc}")
                nc.sync.dma_start(
                    xin[:],
                    gX[qc].rearrange("x (a p) t -> p x a t", p=128),
                )
                for (w_sb, b_sb, dst, ti) in ((wq_sb, bq_sb, QT_sb, 0),
                                              (wk_sb, bk_sb, KT_sb, 1)):
                    for m in range(2):
                        pq = psS.tile([128, 1024], F32, tag="sc",
                                      name=f"qkps_{qc}_{ti}_{m}")
                        for kt in range(8):
                            nc.tensor.matmul(
                                pq[:, 0:TPS],
                                w_sb[:, kt, m * 128:(m + 1) * 128],
                                xin[:, ti, kt, :],
                                start=(kt == 0), stop=(kt == 7),
                            )
                        nc.vector.tensor_scalar_add(
                            dst[:, m, qc * TPS:(qc + 1) * TPS], pq[:, 0:TPS],
                            b_sb[:, m:m + 1],
                        )
                for tsub in range(4):
                    tt = qc * 4 + tsub
                    pv = psS.tile([128, 1024], F32, tag="sc",
                                  name=f"vps_{qc}_{tsub}")
                    for kt in range(8):
                        nc.tensor.matmul(
                            pv[:, 0:COF],
                            xin[:, 2, kt, tsub * 128:(tsub + 1) * 128],
                            wv_sb[:, kt, :],
                            start=(kt == 0), stop=(kt == 7),
                        )
                    nc.vector.tensor_add(
                        V_sb[:, tt, :, 0:64],
                        pv[:, 0:COF].rearrange("p (h c) -> p h c", h=HPC),
                        bv_bc[:],
                    )

            # ---- attention (baseline structure, f32r internals) ----
            def att_pass_alloc(hp, qh):
                return [[psA.tile([128, 512], F32, tag="ps",
                                  name=f"po_{hp}_{qh}_{h2}_{qcl}")
                         for qcl in range(2)] for h2 in range(2)]

            def att_ktgroup(hp, qh, po, kts):
                for kt in kts:
                    for h2 in range(2):
                        p0 = h2 * 64
                        sc = psS.tile([128, 1024], F32, tag="sc",
                                      name=f"sc_{hp}_{qh}_{kt}_{h2}")
                        for qcl in range(2):
                            qg = qh * 2 + qcl
                            nc.tensor.matmul(
                                sc[:, qcl * 512:(qcl + 1) * 512],
                                KT_sb[p0:p0 + 64, hp, kt * 128:(kt + 1) * 128],
                                QT_sb[p0:p0 + 64, hp, qg * 512:(qg + 1) * 512],
                                start=True, stop=True,
                                tile_position=(p0, 0),
                            )
                        ex = expp.tile([128, 1024], F32R, tag="ex",
                                       name=f"ex_{hp}_{qh}_{kt}_{h2}")
                        nc.scalar.activation(out=ex[:], in_=sc[:], func=EXP,
                                             scale=0.125)
                        for qcl in range(2):
                            nc.tensor.matmul(
                                po[h2][qcl][0:65, :],
                                V_sb[:, kt, hp * 2 + h2, :],
                                ex[:, qcl * 512:(qcl + 1) * 512],
                                start=(kt == 0), stop=(kt == 15),
                            )

            def att_norm(hp, qh, po):
                # OT = po[0:64] / l  (l rides in po row 64)
                for h2 in range(2):
                    for qcl in range(2):
                        qg = qh * 2 + qcl
                        p = po[h2][qcl]
                        linv = small.tile([1, 512], F32R, tag="linv",
                                          name=f"linv_{hp}_{qh}_{h2}_{qcl}")
                        nc.vector.reciprocal(linv[:], p[64:65, :])
                        bc_ps = psS.tile([64, 512], F32, tag="sc",
                                         name=f"bc_{hp}_{qh}_{h2}_{qcl}")
                        nc.tensor.matmul(
                            bc_ps[:], ones_sb[:], linv[:],
                            start=True, stop=True,
                        )
                        bc_sb = bcp.tile([64, 512], F32, tag="bc",
                                         name=f"bcs_{hp}_{qh}_{h2}_{qcl}")
                        nc.vector.tensor_copy(bc_sb[:], bc_ps[:])
                        nc.vector.tensor_mul(
                            OT_sb[h2 * 64:(h2 + 1) * 64, hp,
                                  qg * 512:(qg + 1) * 512],
                            p[0:64, :], bc_sb[:],
                        )

            def outproj_half(qh):
                # token-major partial: out[t, of] = OT[:, t].T @ wo  (256 feats)
                for tb in range(8):
                    tok0 = qh * 1024 + tb * 128
                    pg = [psA.tile([128, 512], F32, tag="ps",
                                   name=f"pg_{qh}_{tb}_{i}") for i in range(2)]
                    for ct in range(2):
                        for i in range(2):
                            nc.tensor.matmul(
                                pg[i][:],
                                OT_sb[:, ct, tok0:tok0 + 128],
                                wo_sb[:, ct, i * 512:(i + 1) * 512],
                                start=(ct == 0), stop=(ct == 1),
                            )
                    st = stage_pool.tile([128, 1024], F16, tag="st",
                                         name=f"st_{qh}_{tb}")
                    for i in range(2):
                        nc.vector.tensor_copy(st[:, i * 512:(i + 1) * 512],
                                              pg[i][:])
                    nc.sync.dma_start(ob_in[tok0:tok0 + 128, :], st[:])

            # ---- schedule (sequential; tunnel dominates, not device) ----
            for qc in range(4):
                proj_chunk(qc)
            for qh in range(2):
                for hp in range(2):
                    po = att_pass_alloc(hp, qh)
                    att_ktgroup(hp, qh, po, range(16))
                    att_norm(hp, qh, po)
                outproj_half(qh)

            nc.gpsimd.collective_compute(
                "ReduceScatter",
                mybir.AluOpType.add,
                replica_groups=GROUPS4,
                ins=[ob_in[:].opt()],
                outs=[ob_out[:].opt()],
            )
            nc.sync.dma_start(outp[:], ob_out[:])

    nc.compile()
    return nc


def _get_runner():
    """Build the bass program and a cached jitted PJRT executable once."""
    global _CACHED
    if _CACHED is not None:
        return _CACHED

    import jax
    from jax.sharding import Mesh, PartitionSpec
    from jax.experimental.shard_map import shard_map
    from concourse import mybir
    from concourse.bass2jax import (_bass_exec_p, install_neuronx_cc_hook,
                                    partition_id_tensor)

    nc = _build()
    install_neuronx_cc_hook()

    partition_name = (nc.partition_id_tensor.name
                      if nc.partition_id_tensor else None)
    in_names, out_names, out_avals, zero_shapes = [], [], [], []
    for alloc in nc.m.functions[0].allocations:
        if not isinstance(alloc, mybir.MemoryLocationSet):
            continue
        name = alloc.memorylocations[0].name
        if alloc.kind == "ExternalInput":
            if name != partition_name:
                in_names.append(name)
        elif alloc.kind == "ExternalOutput":
            shape = tuple(alloc.tensor_shape)
            dtype = mybir.dt.np(alloc.dtype)
            out_names.append(name)
            out_avals.append(jax.core.ShapedArray(shape, dtype))
            zero_shapes.append(((N_CORES * shape[0],) + shape[1:], dtype))
    n_params = len(in_names)
    n_outs = len(out_names)
    in_names_all = in_names + out_names + (
        [partition_name] if partition_name else [])

    def _body(*args):
        operands = list(args)
        if partition_name is not None:
            operands.append(partition_id_tensor())
        outs = _bass_exec_p.bind(
            *operands, out_avals=tuple(out_avals),
            in_names=tuple(in_names_all), out_names=tuple(out_names),
            lowering_input_output_aliases=(), sim_require_finite=True,
            sim_require_nnan=True, nc=nc)
        return tuple(outs)

    devices = jax.devices()[:N_CORES]
    mesh = Mesh(np.asarray(devices), ("core",))
    in_specs = (PartitionSpec("core"),) * (n_params + n_outs)
    out_specs = (PartitionSpec("core"),) * n_outs
    donate = tuple(range(n_params, n_params + n_outs))
    sharded = jax.jit(shard_map(_body, mesh=mesh, in_specs=in_specs,
                                out_specs=out_specs, check_rep=False),
                      donate_argnums=donate, keep_unused=True)

    _CACHED = dict(sharded=sharded, in_names=in_names,
                   zero_shapes=zero_shapes, out_names=out_names)
    return _CACHED


def kernel(q, k, v, w_q, b_q, w_k, b_k, w_v, b_v, w_o, b_o):
    import jax

    q, k, v = (np.asarray(x, np.float32) for x in (q, k, v))
    w_q, b_q, w_k, b_k, w_v, b_v, w_o, b_o = (
        np.asarray(x, np.float32)
        for x in (w_q, b_q, w_k, b_k, w_v, b_v, w_o, b_o)
    )

    r = _get_runner()

    # Concatenated per-core inputs (core axis leading, then per-core shape).
    # Core c: batch b=c//4, head-group hg=c%4, token shard ts=c%4.
    f16 = np.float16
    xs = {}
    for nm, x in (("xq", q), ("xk", k), ("xv", v)):
        a = np.empty((2, 4, D_MODEL, TPS), f16)
        a[:] = x.reshape(2, 4, TPS, D_MODEL).transpose(0, 1, 3, 2)
        xs[nm] = a.reshape(N_CORES * D_MODEL, TPS)
    for nm, w in (("wq", w_q), ("wk", w_k), ("wv", w_v)):
        a = np.empty((2, 4, D_MODEL, COF), f16)
        a[:] = w.reshape(D_MODEL, 4, COF).transpose(1, 0, 2)
        xs[nm] = a.reshape(N_CORES * D_MODEL, COF)
    a = np.empty((2, 4, COF, D_MODEL), f16)
    a[:] = w_o.reshape(4, COF, D_MODEL)
    xs["wo"] = a.reshape(N_CORES * COF, D_MODEL)
    bq8 = np.empty((2, 4, 128, 2), np.float32)
    bq8[:] = b_q.reshape(4, 2, 128).transpose(0, 2, 1)
    xs["bq2"] = bq8.reshape(N_CORES * 128, 2)
    bk8 = np.empty((2, 4, 128, 2), np.float32)
    bk8[:] = b_k.reshape(4, 2, 128).transpose(0, 2, 1)
    xs["bk2"] = bk8.reshape(N_CORES * 128, 2)
    bv8 = np.empty((2, 4, HPC, 64), np.float32)
    bv8[:] = b_v.reshape(4, HPC, 64)
    xs["bv4"] = bv8.reshape(N_CORES * HPC, 64)
    xs["ones"] = np.ones((N_CORES * 1, 64), np.float32)

    concat_in = [xs[nm] for nm in r["in_names"]]
    concat_zeros = [np.zeros(shape, dt) for shape, dt in r["zero_shapes"]]
    out_arrs = r["sharded"](*concat_in, *concat_zeros)

    # outp per core: [512, 1024] fp16, tokens (c%4)*512.. of batch c//4
    res = np.asarray(out_arrs[0])
    out = res.reshape(2, S, D_MODEL).astype(np.float32)
    out += b_o
    return out


# revision 7
# speedup vs baseline: 17.5707x; 1.4791x over previous
"""Multi-head attention (B=2, S=2048, D=1024, H=16) on 8 Trainium2 NeuronCores.

Sharding: data-parallel over batch (2 groups of 4 cores) x tensor-parallel over
heads (4 heads / core). The wall clock is dominated by the host<->device tunnel,
so the design minimizes transferred bytes and per-call overhead:

  - Inputs ship as fp16 token shards, token-major (no host transpose; the
    device PE-transposes after an on-device AllGather reconstructs the full
    sequence within each 4-core batch group).
  - Weights ship fp16, split in half between paired cores (c, c+4) and
    reassembled with a 2-core AllGather — every weight byte crosses the
    tunnel exactly once.
  - All inputs are packed into 4 arrays (xqkv / w / b / idm) to amortize
    per-array dispatch+transfer overhead; host staging buffers are
    preallocated once and reused (no per-call page faults).
  - Each core's partial output projection is summed on device with a
    ReduceScatter; each core returns its 512 tokens of the final output in
    fp16. The donated output buffer from the previous call is recycled so
    no zero-buffer is uploaded.
  - The jitted PJRT executable is built once and cached.

Device kernel notes (per core):
  - Raw token-major fp16 chunks are transposed feature-major via PE identity
    matmuls (psum f32 -> fp16 copy), then projections consume fp16 operands;
    attention internals stay f32r (scores computed transposed, softmax
    without max-subtraction, denominator via a ones-column in the PV
    stationary operand, 1/l broadcast with a K=1 ones matmul).
  - The output projection is computed token-major so ReduceScatter chunks
    are token-contiguous and host reassembly is a plain cast.
"""

import numpy as np

D_MODEL = 1024
S = 2048
N_CORES = 8
HPC = 4           # heads per core
COF = HPC * 64    # 256 out-features per core
TPS = S // 4      # 512 tokens per shard
WBLK = D_MODEL * COF  # 262144 elems per weight slice
WHALF = 2 * WBLK      # per-core weight half

_CACHED = None


def _build():
    from concourse import bacc
    import concourse.bass as bass
    import concourse.tile as tile
    from concourse import mybir

    F16 = mybir.dt.float16
    F32R = mybir.dt.float32r
    F32 = mybir.dt.float32
    EXP = mybir.ActivationFunctionType.Exp

    nc = bacc.Bacc("TRN2", target_bir_lowering=False, debug=False,
                   num_devices=N_CORES)

    xqkv = nc.dram_tensor("xqkv", [3, TPS, D_MODEL], F16, kind="ExternalInput")
    w_in = nc.dram_tensor("w_in", [WHALF], F16, kind="ExternalInput")
    b_in = nc.dram_tensor("b_in", [832], F32R, kind="ExternalInput")
    idm = nc.dram_tensor("idm", [128, 128], F16, kind="ExternalInput")
    outp = nc.dram_tensor("outp", [TPS, D_MODEL], F16, kind="ExternalOutput")

    GROUPS4 = [[0, 1, 2, 3], [4, 5, 6, 7]]
    PAIRS = [[0, 4], [1, 5], [2, 6], [3, 7]]

    with nc.allow_low_precision(reason="fp16 transfers / f32r matmuls intended"), \
            tile.TileContext(nc) as tc:
        with (
            tc.tile_pool(name="dram", bufs=1, space="DRAM") as dram,
            tc.tile_pool(name="wconst", bufs=1) as wconst,
            tc.tile_pool(name="big", bufs=1) as big,
            tc.tile_pool(name="raw", bufs=2) as raw_pool,
            tc.tile_pool(name="qin", bufs=2) as qin_pool,
            tc.tile_pool(name="expp", bufs=4) as expp,
            tc.tile_pool(name="stage", bufs=3) as stage_pool,
            tc.tile_pool(name="bcp", bufs=2) as bcp,
            tc.tile_pool(name="small", bufs=4) as small,
            tc.tile_pool(name="psA", bufs=4, space="PSUM") as psA,
            tc.tile_pool(name="psS", bufs=2, space="PSUM") as psS,
        ):
            # ---- collective bounce buffers (DRAM) ----
            xin_b = dram.tile([3, TPS, D_MODEL], F16)    # my shard of q,k,v
            gX = dram.tile([4, 3, TPS, D_MODEL], F16)    # gathered full seq
            wb = dram.tile([WHALF], F16)                 # my weight half
            gW = dram.tile([2, WHALF], F16)              # full weight block
            ob_in = dram.tile([S, D_MODEL], F16)         # my partial out
            ob_out = dram.tile([TPS, D_MODEL], F16)      # reduced shard

            nc.sync.dma_start(xin_b[:], xqkv[:])
            nc.sync.dma_start(wb[:], w_in[:])
            nc.gpsimd.collective_compute(
                "AllGather", mybir.AluOpType.bypass,
                replica_groups=GROUPS4,
                ins=[xin_b[:].opt()], outs=[gX[:].opt()],
            )
            nc.gpsimd.collective_compute(
                "AllGather", mybir.AluOpType.bypass,
                replica_groups=PAIRS,
                ins=[wb[:].opt()], outs=[gW[:].opt()],
            )

            # ---- weights / biases to SBUF ----
            wq_sb = wconst.tile([128, 8, COF], F16)
            wk_sb = wconst.tile([128, 8, COF], F16)
            wv_sb = wconst.tile([128, 8, COF], F16)
            wo_sb = wconst.tile([128, 2, D_MODEL], F16)
            nc.sync.dma_start(
                wq_sb[:], gW[0, 0:WBLK].rearrange("(a p f) -> p a f", p=128, f=COF))
            nc.sync.dma_start(
                wk_sb[:], gW[0, WBLK:WHALF].rearrange("(a p f) -> p a f", p=128, f=COF))
            nc.sync.dma_start(
                wv_sb[:], gW[1, 0:WBLK].rearrange("(a p f) -> p a f", p=128, f=COF))
            nc.sync.dma_start(
                wo_sb[:], gW[1, WBLK:WHALF].rearrange("(c p f) -> p c f", p=128, f=D_MODEL))

            bq_sb = wconst.tile([128, 2], F32)
            bk_sb = wconst.tile([128, 2], F32)
            nc.sync.dma_start(
                bq_sb[:], b_in[0:256].rearrange("(p m) -> p m", m=2).bitcast(F32))
            nc.sync.dma_start(
                bk_sb[:], b_in[256:512].rearrange("(p m) -> p m", m=2).bitcast(F32))
            b_ap = b_in[:]
            bv_bc = wconst.tile([128, HPC, 64], F32)
            nc.gpsimd.dma_start(
                bv_bc[:],
                bass.AP(tensor=b_ap.tensor, offset=b_ap.offset + 512,
                        ap=[[0, 128], [64, HPC], [1, 64]]).bitcast(F32),
            )
            ones_sb = wconst.tile([1, 64], F32R)
            nc.sync.dma_start(ones_sb[:],
                              b_in[768:832].rearrange("(o c) -> o c", o=1))
            id_sb = wconst.tile([128, 128], F16)
            nc.sync.dma_start(id_sb[:], idm[:])

            # ---- persistent activations ----
            QT_sb = big.tile([128, 2, S], F32R)   # [p, m, t]: Q^T[m*128+p, t]
            KT_sb = big.tile([128, 2, S], F32R)
            V_sb = big.tile([128, 16, HPC, 65], F32R)  # [tok%128, tok//128, h, c]
            OT_sb = big.tile([128, 2, S], F16)    # normalized attention out^T

            # V ones-column (l accumulator rides along the PV matmul)
            for tt in range(16):
                nc.gpsimd.dma_start(
                    V_sb[:, tt, :, 64:65],
                    bass.AP(tensor=b_ap.tensor, offset=b_ap.offset + 768,
                            ap=[[0, 128], [0, HPC], [1, 1]]),
                )

            # ---- per-chunk: PE-transpose raw tokens, then project ----
            def proj_chunk(qc):
                # raw [tok%128, tokblk, featblk, feat] per tensor
                xin = qin_pool.tile([128, 3, 8, TPS], F16, tag="xin",
                                    name=f"xin_{qc}")
                for t in range(3):
                    rw = raw_pool.tile([128, 4, 8, 128], F16, tag="raw",
                                       name=f"raw_{qc}_{t}")
                    nc.sync.dma_start(
                        rw[:],
                        gX[qc, t].rearrange("(tb p) (fb f) -> p tb fb f",
                                            p=128, f=128),
                    )
                    for fb in range(8):
                        ps = psA.tile([128, 512], F32, tag="ps",
                                      name=f"tp_{qc}_{t}_{fb}")
                        for tb in range(4):
                            nc.tensor.matmul(
                                ps[:, tb * 128:(tb + 1) * 128],
                                rw[:, tb, fb, :], id_sb[:],
                                start=True, stop=True,
                            )
                        nc.vector.tensor_copy(xin[:, t, fb, :], ps[:])
                # Q/K projections (feature-major psum)
                for (ti, b_sb, dst) in ((0, bq_sb, QT_sb), (1, bk_sb, KT_sb)):
                    w_sb = wq_sb if ti == 0 else wk_sb
                    for m in range(2):
                        pq = psS.tile([128, 1024], F32, tag="sc",
                                      name=f"qkps_{qc}_{ti}_{m}")
                        for kt in range(8):
                            nc.tensor.matmul(
                                pq[:, 0:TPS],
                                w_sb[:, kt, m * 128:(m + 1) * 128],
                                xin[:, ti, kt, :],
                                start=(kt == 0), stop=(kt == 7),
                            )
                        nc.vector.tensor_scalar_add(
                            dst[:, m, qc * TPS:(qc + 1) * TPS], pq[:, 0:TPS],
                            b_sb[:, m:m + 1],
                        )
                # V projection (token-major psum)
                for tsub in range(4):
                    tt = qc * 4 + tsub
                    pv = psS.tile([128, 1024], F32, tag="sc",
                                  name=f"vps_{qc}_{tsub}")
                    for kt in range(8):
                        nc.tensor.matmul(
                            pv[:, 0:COF],
                            xin[:, 2, kt, tsub * 128:(tsub + 1) * 128],
                            wv_sb[:, kt, :],
                            start=(kt == 0), stop=(kt == 7),
                        )
                    nc.vector.tensor_add(
                        V_sb[:, tt, :, 0:64],
                        pv[:, 0:COF].rearrange("p (h c) -> p h c", h=HPC),
                        bv_bc[:],
                    )

            # ---- attention (baseline structure, f32r internals) ----
            def att_pass_alloc(hp, qh):
                return [[psA.tile([128, 512], F32, tag="ps",
                                  name=f"po_{hp}_{qh}_{h2}_{qcl}")
                         for qcl in range(2)] for h2 in range(2)]

            def att_ktgroup(hp, qh, po, kts):
                for kt in kts:
                    for h2 in range(2):
                        p0 = h2 * 64
                        sc = psS.tile([128, 1024], F32, tag="sc",
                                      name=f"sc_{hp}_{qh}_{kt}_{h2}")
                        for qcl in range(2):
                            qg = qh * 2 + qcl
                            nc.tensor.matmul(
                                sc[:, qcl * 512:(qcl + 1) * 512],
                                KT_sb[p0:p0 + 64, hp, kt * 128:(kt + 1) * 128],
                                QT_sb[p0:p0 + 64, hp, qg * 512:(qg + 1) * 512],
                                start=True, stop=True,
                                tile_position=(p0, 0),
                            )
                        ex = expp.tile([128, 1024], F32R, tag="ex",
                                       name=f"ex_{hp}_{qh}_{kt}_{h2}")
                        nc.scalar.activation(out=ex[:], in_=sc[:], func=EXP,
                                             scale=0.125)
                        for qcl in range(2):
                            nc.tensor.matmul(
                                po[h2][qcl][0:65, :],
                                V_sb[:, kt, hp * 2 + h2, :],
                                ex[:, qcl * 512:(qcl + 1) * 512],
                                start=(kt == 0), stop=(kt == 15),
                            )

            def att_norm(hp, qh, po):
                # OT = po[0:64] / l  (l rides in po row 64)
                for h2 in range(2):
                    for qcl in range(2):
                        qg = qh * 2 + qcl
                        p = po[h2][qcl]
                        linv = small.tile([1, 512], F32R, tag="linv",
                                          name=f"linv_{hp}_{qh}_{h2}_{qcl}")
                        nc.vector.reciprocal(linv[:], p[64:65, :])
                        bc_ps = psS.tile([64, 512], F32, tag="sc",
                                         name=f"bc_{hp}_{qh}_{h2}_{qcl}")
                        nc.tensor.matmul(
                            bc_ps[:], ones_sb[:], linv[:],
                            start=True, stop=True,
                        )
                        bc_sb = bcp.tile([64, 512], F32, tag="bc",
                                         name=f"bcs_{hp}_{qh}_{h2}_{qcl}")
                        nc.vector.tensor_copy(bc_sb[:], bc_ps[:])
                        nc.vector.tensor_mul(
                            OT_sb[h2 * 64:(h2 + 1) * 64, hp,
                                  qg * 512:(qg + 1) * 512],
                            p[0:64, :], bc_sb[:],
                        )

            def outproj_half(qh):
                # token-major partial: out[t, of] = OT[:, t].T @ wo  (256 feats)
                for tb in range(8):
                    tok0 = qh * 1024 + tb * 128
                    pg = [psA.tile([128, 512], F32, tag="ps",
                                   name=f"pg_{qh}_{tb}_{i}") for i in range(2)]
                    for ct in range(2):
                        for i in range(2):
                            nc.tensor.matmul(
                                pg[i][:],
                                OT_sb[:, ct, tok0:tok0 + 128],
                                wo_sb[:, ct, i * 512:(i + 1) * 512],
                                start=(ct == 0), stop=(ct == 1),
                            )
                    st = stage_pool.tile([128, 1024], F16, tag="st",
                                         name=f"st_{qh}_{tb}")
                    for i in range(2):
                        nc.vector.tensor_copy(st[:, i * 512:(i + 1) * 512],
                                              pg[i][:])
                    nc.sync.dma_start(ob_in[tok0:tok0 + 128, :], st[:])

            # ---- schedule (sequential; tunnel dominates, not device) ----
            for qc in range(4):
                proj_chunk(qc)
            for qh in range(2):
                for hp in range(2):
                    po = att_pass_alloc(hp, qh)
                    att_ktgroup(hp, qh, po, range(16))
                    att_norm(hp, qh, po)
                outproj_half(qh)

            nc.gpsimd.collective_compute(
                "ReduceScatter", mybir.AluOpType.add,
                replica_groups=GROUPS4,
                ins=[ob_in[:].opt()], outs=[ob_out[:].opt()],
            )
            nc.sync.dma_start(outp[:], ob_out[:])

    nc.compile()
    return nc


def _get_runner():
    """Build the bass program, cached jitted PJRT executable, and reusable
    host staging buffers once."""
    global _CACHED
    if _CACHED is not None:
        return _CACHED

    import jax
    from jax.sharding import Mesh, PartitionSpec
    from jax.experimental.shard_map import shard_map
    from concourse import mybir
    from concourse.bass2jax import (_bass_exec_p, install_neuronx_cc_hook,
                                    partition_id_tensor)

    nc = _build()
    install_neuronx_cc_hook()

    partition_name = (nc.partition_id_tensor.name
                      if nc.partition_id_tensor else None)
    in_names, out_names, out_avals, zero_shapes = [], [], [], []
    for alloc in nc.m.functions[0].allocations:
        if not isinstance(alloc, mybir.MemoryLocationSet):
            continue
        name = alloc.memorylocations[0].name
        if alloc.kind == "ExternalInput":
            if name != partition_name:
                in_names.append(name)
        elif alloc.kind == "ExternalOutput":
            shape = tuple(alloc.tensor_shape)
            dtype = mybir.dt.np(alloc.dtype)
            out_names.append(name)
            out_avals.append(jax.core.ShapedArray(shape, dtype))
            zero_shapes.append(((N_CORES * shape[0],) + shape[1:], dtype))
    n_params = len(in_names)
    n_outs = len(out_names)
    in_names_all = in_names + out_names + (
        [partition_name] if partition_name else [])

    def _body(*args):
        operands = list(args)
        if partition_name is not None:
            operands.append(partition_id_tensor())
        outs = _bass_exec_p.bind(
            *operands, out_avals=tuple(out_avals),
            in_names=tuple(in_names_all), out_names=tuple(out_names),
            lowering_input_output_aliases=(), sim_require_finite=True,
            sim_require_nnan=True, nc=nc)
        return tuple(outs)

    devices = jax.devices()[:N_CORES]
    mesh = Mesh(np.asarray(devices), ("core",))
    in_specs = (PartitionSpec("core"),) * (n_params + n_outs)
    out_specs = (PartitionSpec("core"),) * n_outs
    donate = tuple(range(n_params, n_params + n_outs))
    sharded = jax.jit(shard_map(_body, mesh=mesh, in_specs=in_specs,
                                out_specs=out_specs, check_rep=False),
                      donate_argnums=donate, keep_unused=True)

    # preallocated host staging buffers (reused across calls)
    f16 = np.float16
    stage = {
        "xq8": np.zeros((2, 4, 3, TPS, D_MODEL), f16),
        "w8": np.zeros((2, 4, WHALF), f16),
        "b8": np.zeros((2, 4, 832), np.float32),
        "id8": np.zeros((N_CORES * 128, 128), f16),
    }
    stage["id8"].reshape(N_CORES, 128, 128)[:] = np.eye(128, dtype=f16)

    _CACHED = dict(sharded=sharded, in_names=in_names,
                   zero_shapes=zero_shapes, out_names=out_names,
                   stage=stage, prev_out=None)
    return _CACHED


def kernel(q, k, v, w_q, b_q, w_k, b_k, w_v, b_v, w_o, b_o):
    q, k, v = (np.asarray(x, np.float32) for x in (q, k, v))
    w_q, b_q, w_k, b_k, w_v, b_v, w_o, b_o = (
        np.asarray(x, np.float32)
        for x in (w_q, b_q, w_k, b_k, w_v, b_v, w_o, b_o)
    )

    r = _get_runner()
    st = r["stage"]

    # xqkv: [core=(b,ts)][3, 512, 1024] token-major fp16
    xq8 = st["xq8"]
    xq8[:, :, 0] = q.reshape(2, 4, TPS, D_MODEL)
    xq8[:, :, 1] = k.reshape(2, 4, TPS, D_MODEL)
    xq8[:, :, 2] = v.reshape(2, 4, TPS, D_MODEL)

    # weights: full block per head group = [wq_sl|wk_sl|wv_sl|wo_sl] flat;
    # core c gets half c//4 of its head group's block
    w8 = st["w8"]
    wf = w8.reshape(2, 4, 2, WBLK)  # [half, hg, (sub-half of pair), WBLK]
    # half 0 of the pair = wq|wk, half 1 = wv|wo
    wf[0, :, 0].reshape(4, D_MODEL, COF)[:] = (
        w_q.reshape(D_MODEL, 4, COF).transpose(1, 0, 2))
    wf[0, :, 1].reshape(4, D_MODEL, COF)[:] = (
        w_k.reshape(D_MODEL, 4, COF).transpose(1, 0, 2))
    wf[1, :, 0].reshape(4, D_MODEL, COF)[:] = (
        w_v.reshape(D_MODEL, 4, COF).transpose(1, 0, 2))
    wf[1, :, 1].reshape(4, COF, D_MODEL)[:] = w_o.reshape(4, COF, D_MODEL)

    # biases: [0:256] bq p-major, [256:512] bk, [512:768] bv, [768:832] ones
    b8 = st["b8"]
    b8[:, :, 0:256].reshape(2, 4, 128, 2)[:] = (
        b_q.reshape(4, 2, 128).transpose(0, 2, 1))
    b8[:, :, 256:512].reshape(2, 4, 128, 2)[:] = (
        b_k.reshape(4, 2, 128).transpose(0, 2, 1))
    b8[:, :, 512:768] = b_v.reshape(4, 256)
    b8[:, :, 768:832] = 1.0

    xs = {
        "xqkv": xq8.reshape(N_CORES * 3, TPS, D_MODEL),
        "w_in": w8.reshape(N_CORES * WHALF),
        "b_in": b8.reshape(N_CORES * 832),
        "idm": st["id8"],
    }
    concat_in = [xs[nm] for nm in r["in_names"]]
    if r["prev_out"] is None:
        donated = [np.zeros(shape, dt) for shape, dt in r["zero_shapes"]]
    else:
        donated = [r["prev_out"]]
    out_arrs = r["sharded"](*concat_in, *donated)
    r["prev_out"] = out_arrs[0]

    # fetch shards in parallel; shard c = tokens (c%4)*512.. of batch c//4
    out = np.empty((2, S, D_MODEL), np.float32)
    out4 = out.reshape(N_CORES, TPS, D_MODEL)
    shards = sorted(out_arrs[0].addressable_shards,
                    key=lambda sh: sh.index[0].start or 0)
    for sh in shards:
        sh.data.copy_to_host_async()
    for i, sh in enumerate(shards):
        out4[i] = np.asarray(sh.data)
    out += b_o
    return out
